# revision 1
# baseline (speedup 1.0000x reference)
"""Cantor cross-attention Trainium2 kernel.

Sharding: 8 cores = (batch b = core//4) x (4 heads = 4*(core%4)..+4).
Each core computes its 4 heads' attention + partial output projection
(partial^T [1024, 2048]); host sums 4 partials per batch and adds bo.

Dataflow (per head, transposed layout S^T[sj_chunk(128 part), si(free)]):
  scores^T = K^T.T @ Q^T (f32r matmuls, scale folded into Wq)
  psum += mask_bias (DVE, bias = 0 allowed / -64 masked, bf16)
  P^T = exp(psum) (ACT -> f32r SBUF; masked -> e^-64 ~ 0)
  out^T[65, si] = sum_sj [V|1]^T P^T  (f32r PV, K=128; row 64 = denom)
  out = psum[0:64] * recip(denom broadcast)  (DVE)
Static Cantor mask is compacted to active 256-wide si-subwindows per
sj-chunk (bank-aligned matmul units, ~83% of columns).
"""

import numpy as np
import ml_dtypes

import concourse.bacc as bacc
import concourse.mybir as mybir
from concourse import tile

F32 = mybir.dt.float32
F32R = mybir.dt.float32r
BF16 = mybir.dt.bfloat16
FP8 = mybir.dt.float8e4
IDENT = mybir.ActivationFunctionType.Identity
EXP = mybir.ActivationFunctionType.Exp

S, D, H, HD = 2048, 1024, 16, 64
DEPTH, LOCAL_W = 7, 64
SCALE = 1.0 / HD ** 0.5
NCH = S // 128          # 16 sj chunks
NG = 2                  # head groups per core (2 heads each)
HPC = 4                 # heads per core
MASK_BIAS = -64.0


# ---------------------------------------------------------------- host plan

def _cantor_mask():
    idx = np.arange(S)
    d = np.abs(idx[:, None] - idx[None, :])
    x = d.copy()
    ok = np.ones_like(d, dtype=bool)
    for _ in range(DEPTH):
        ok &= (x % 3) != 1
        x //= 3
    ok &= x == 0
    return ok | (d <= LOCAL_W)


def _plan():
    """Per sj-chunk: active 256-wide si-subwindows. Every matmul unit is one
    subwindow (width 256, si- and compact-offset 256-aligned, never crosses
    a PSUM bank). Pieces = compact 512-blocks (1 bank) of 1-2 units."""
    mask = _cantor_mask()
    chunks = []
    for c in range(NCH):
        act = mask[c * 128:(c + 1) * 128].any(axis=0).reshape(8, 256).any(axis=1)
        subw = [int(s) for s in np.where(act)[0]]
        units = [(256 * s, 256, 256 * i) for i, s in enumerate(subw)]
        pieces = []
        for p0 in range(0, len(units), 4):
            us = list(range(p0, min(p0 + 4, len(units))))
            pieces.append((units[us[0]][2], 256 * len(us), us))
        chunks.append({"units": units, "pieces": pieces, "W": 256 * len(units)})
    wmax = max(ch["W"] for ch in chunks)
    mmult = np.zeros((128, NCH, wmax), np.float32)
    for c, ch in enumerate(chunks):
        rows = mask[c * 128:(c + 1) * 128]
        for s0, w, co in ch["units"]:
            mmult[:, c, co:co + w] = rows[:, s0:s0 + w].astype(np.float32)
    return chunks, wmax, mmult


_PLAN = None


def _plan_cached():
    global _PLAN
    if _PLAN is None:
        _PLAN = _plan()
    return _PLAN


# ---------------------------------------------------------------- bass build

def build_nc():
    chunks, WMAX, _ = _plan_cached()
    last_w = {}  # psum bank (si//512) -> (chunk, si0) of its last accumulate
    for c in range(NCH):
        for (s0, w, co) in chunks[c]["units"]:
            last_w[s0 // 512] = (c, s0)
    nc = bacc.Bacc("TRN2", target_bir_lowering=False, debug=False)

    xq = nc.dram_tensor("xq", [D, S], F32R, kind="ExternalInput")      # query[b].T
    xkv = nc.dram_tensor("xkv", [D, S], F32R, kind="ExternalInput")    # key_value[b].T
    wq = nc.dram_tensor("wq", [128, 8, 256], F32R, kind="ExternalInput")
    wkv = nc.dram_tensor("wkv", [128, 8, 512], F32R, kind="ExternalInput")
    wo = nc.dram_tensor("wo", [128, 2, 1024], F32R, kind="ExternalInput")
    bq = nc.dram_tensor("bq", [128, 2], F32, kind="ExternalInput")     # ACT bias
    bkv = nc.dram_tensor("bkv", [1, 512], F32R, kind="ExternalInput")  # K=1 bias row
    mtb_d = nc.dram_tensor("mtb", [128, NCH, WMAX], FP8, kind="ExternalInput")
    cst = nc.dram_tensor("cst", [1, 512], F32R, kind="ExternalInput")
    # cst layout: [0:128]=0.0, [128:256]=1.0
    idn = nc.dram_tensor("idn", [128, 128], F32R, kind="ExternalInput")
    dscr = nc.dram_tensor("dscr", [4, S], F32, kind="Internal")
    out = nc.dram_tensor("out", [8, 128, S], F32, kind="ExternalOutput")

    with tile.TileContext(nc) as tc:
        with tc.tile_pool(name="consts", bufs=1) as cp, \
             tc.tile_pool(name="persist", bufs=1) as pp:
            wq_t = cp.tile([128, 8, 256], F32R)
            wkv_t = cp.tile([128, 8, 512], F32R)
            wo_t = cp.tile([128, 2, 1024], F32R)
            bq_t = cp.tile([128, 2], F32)
            bkv_t = cp.tile([1, 512], F32R)
            cst_t = cp.tile([1, 512], F32R)
            idn_t = cp.tile([128, 128], F32R)
            for dst, src in ((wkv_t, wkv), (bkv_t, bkv), (cst_t, cst),
                             (idn_t, idn)):
                nc.sync.dma_start(dst[:], src.ap())
            ones128 = cst_t[:, 128:256]
            zeros65 = cst_t[:, 0:65]

            qt = [pp.tile([128, S], F32R, name=f"qt{g}") for g in range(NG)]
            kt = [pp.tile([128, S], F32R, name=f"kt{g}") for g in range(NG)]
            vbn = [pp.tile([128, 260], F32R, name=f"vbn{c}") for c in range(NCH)]
            oa = [pp.tile([128, S], F32R, name=f"oa{g}") for g in range(NG)]
            mtb = [pp.tile([128, WMAX], FP8, name=f"mtb{c}") for c in range(NCH)]

            # ---- phase 1a: K,V natural (si-half x dchunk-outer) ----
            for half in (0, 1):
                with tc.tile_pool(name=f"kn{half}", bufs=8) as knp:
                    kns = []
                    with tc.tile_pool(name=f"xkv{half}", bufs=4) as xs, \
                         tc.tile_pool(name=f"pkv{half}", bufs=1, space="PSUM") as pkv:
                        pskv = [pkv.tile([128, 512], F32, name=f"pskv{half}_{st}",
                                         tag=f"kv{st}") for st in range(8)]
                        for dc in range(8):
                            xt = xs.tile([128, 1024], F32R,
                                         name=f"xkv{half}_{dc}", tag="x")
                            nc.sync.dma_start(
                                xt[:], xkv.ap()[dc * 128:(dc + 1) * 128,
                                                half * 1024:(half + 1) * 1024])
                            for st in range(8):
                                nc.tensor.matmul(pskv[st][:],
                                                 xt[:, st * 128:(st + 1) * 128],
                                                 wkv_t[:, dc, :],
                                                 start=(dc == 0), stop=False)
                        for st in range(8):
                            sg = half * 8 + st
                            nc.tensor.matmul(pskv[st][:], ones128, bkv_t[:],
                                             start=False, stop=True)
                            kn = knp.tile([128, 256], F32R, name=f"kn{sg}",
                                          tag="kn")
                            nc.vector.tensor_copy(kn[:], pskv[st][:, 0:256])
                            nc.vector.tensor_copy(
                                vbn[sg][:].rearrange("p (h c) -> p h c",
                                                     c=65)[:, :, 0:64],
                                pskv[st][:, 256:512].rearrange(
                                    "p (h c) -> p h c", c=64))
                            kns.append((sg, kn))
                    with tc.tile_pool(name=f"ptp{half}", bufs=2,
                                      space="PSUM") as ptp:
                        for sg, kn in kns:
                            for g in range(NG):
                                pst = ptp.tile([128, 128], F32R,
                                               name=f"pst{sg}_{g}", tag="tp")
                                nc.tensor.transpose(
                                    pst[:], kn[:, g * 128:(g + 1) * 128], idn_t[:])
                                nc.vector.tensor_copy(
                                    kt[g][:, sg * 128:(sg + 1) * 128], pst[:])

            # ---- phase 1c: Q^T groups ----
            nc.sync.dma_start(wq_t[:], wq.ap())
            nc.sync.dma_start(bq_t[:], bq.ap())
            with tc.tile_pool(name="xqp", bufs=4) as xqs, \
                 tc.tile_pool(name="pq", bufs=1, space="PSUM") as pq:
                psq = [pq.tile([128, S], F32, name=f"psq{g}", tag=f"q{g}")
                       for g in range(NG)]
                for dc in range(8):
                    xt = xqs.tile([128, S], F32R, name=f"xq{dc}", tag="x")
                    nc.sync.dma_start(xt[:], xq.ap()[dc * 128:(dc + 1) * 128, :])
                    for g in range(NG):
                        for n in range(4):
                            nc.tensor.matmul(psq[g][:, n * 512:(n + 1) * 512],
                                             wq_t[:, dc, g * 128:(g + 1) * 128],
                                             xt[:, n * 512:(n + 1) * 512],
                                             start=(dc == 0), stop=(dc == 7))
                for g in range(NG):
                    nc.scalar.activation(qt[g][:], psq[g][:], IDENT,
                                         bias=bq_t[:, g:g + 1], scale=1.0)

            for c in range(NCH):
                nc.sync.dma_start(mtb[c][:], mtb_d.ap()[:, c, :])
                # ones columns of [V|1] (col 64 of each 65-block) via DRAM bcast
                nc.sync.dma_start(
                    vbn[c][:].rearrange("p (h c) -> p h c", c=65)[:, :, 64:65],
                    cst.ap()[0:1, 128:132].to_broadcast((128, 4)))

            # ---- phase 3: per-head scores + exp + mask-mul + PV + normalize
            with tc.tile_pool(name="pbp", bufs=6) as pbp, \
                 tc.tile_pool(name="dbp", bufs=1) as dbp, \
                 tc.tile_pool(name="sps", bufs=2, space="PSUM") as sps, \
                 tc.tile_pool(name="bps", bufs=1, space="PSUM") as bps:
                np_tot = 0
                for h in range(HPC):
                    g, r0 = h // 2, 64 * (h % 2)
                    psb = bps.tile([65, S], F32, name=f"psb{h}", tag="psb")
                    for n in range(4):
                        nc.tensor.matmul(psb[:, n * 512:(n + 1) * 512], zeros65,
                                         cst_t[:, 0:512], start=True, stop=False)

                    def term_pv(c, pbs):
                        for ui, (s0, w, co) in enumerate(chunks[c]["units"]):
                            pco = chunks[c]["pieces"][ui // 4][0]
                            nc.tensor.matmul(psb[:, s0:s0 + w],
                                             vbn[c][:, 65 * h:65 * h + 65],
                                             pbs[ui // 4][:, co - pco:co - pco + w],
                                             start=False,
                                             stop=(last_w[s0 // 512] == (c, s0)))

                    pend = []
                    for c in range(NCH):
                        pbs = []
                        for (pco, pw, uis) in chunks[c]["pieces"]:
                            pspc = sps.tile([128, 1024], F32,
                                            name=f"sc{h}_{c}_{pco}", tag="sc")
                            for ui in uis:
                                s0, w, co = chunks[c]["units"][ui]
                                nc.tensor.matmul(
                                    pspc[:, co - pco:co - pco + w],
                                    kt[g][r0:r0 + 64, c * 128:(c + 1) * 128],
                                    qt[g][r0:r0 + 64, s0:s0 + w],
                                    start=True, stop=True)
                            pb = pbp.tile([128, 1024], F32R,
                                          name=f"pb{h}_{c}_{pco}", tag="pb")
                            nc.scalar.activation(pb[:, 0:pw], pspc[:, 0:pw], EXP)
                            eng = nc.vector if np_tot % 3 != 2 else nc.gpsimd
                            eng.tensor_mul(pb[:, 0:pw], pb[:, 0:pw],
                                           mtb[c][:, pco:pco + pw])
                            np_tot += 1
                            pbs.append(pb)
                        pend.append((c, pbs))
                        if len(pend) > 2:
                            term_pv(*pend.pop(0))
                    for cp in pend:
                        term_pv(*cp)

                    # stage psb to SBUF to free the PSUM bank quickly
                    psb_sb = dbp.tile([65, S], F32, name=f"pso{h}", tag="pso",
                                      bufs=2)
                    nc.vector.tensor_copy(psb_sb[:], psb[:])
                    # normalize: out = B * 1/denom (denom = row 64), off-path
                    nc.sync.dma_start(dscr.ap()[h:h + 1, :], psb_sb[64:65, :])
                    for nh in range(2):
                        den_b = dbp.tile([64, 1024], F32, name=f"db{h}_{nh}",
                                         tag="db", bufs=2)
                        nc.sync.dma_start(
                            den_b[:],
                            dscr.ap()[h:h + 1, nh * 1024:(nh + 1) * 1024]
                            .to_broadcast((64, 1024)))
                        nc.vector.reciprocal(den_b[:], den_b[:])
                        nc.vector.tensor_mul(
                            oa[g][r0:r0 + 64, nh * 1024:(nh + 1) * 1024],
                            psb_sb[0:64, nh * 1024:(nh + 1) * 1024], den_b[:])

            # ---- phase 4: output projection ----
            nc.sync.dma_start(wo_t[:], wo.ap())
            with tc.tile_pool(name="osb", bufs=2) as osp, \
                 tc.tile_pool(name="wop", bufs=2, space="PSUM") as wop:
                for dc in range(8):
                    ps = wop.tile([128, S], F32, name=f"pso{dc}", tag="wo")
                    for kc in range(NG):
                        for n in range(4):
                            nc.tensor.matmul(ps[:, n * 512:(n + 1) * 512],
                                             wo_t[:, kc, dc * 128:(dc + 1) * 128],
                                             oa[kc][:, n * 512:(n + 1) * 512],
                                             start=(kc == 0), stop=(kc == NG - 1))
                    for nh in range(2):
                        ob = osp.tile([128, 1024], F32, name=f"ob{dc}_{nh}",
                                      tag="ob")
                        sl = slice(nh * 1024, (nh + 1) * 1024)
                        if (2 * dc + nh) % 2 == 0:
                            nc.scalar.copy(ob[:], ps[:, sl])
                        else:
                            nc.vector.tensor_copy(ob[:], ps[:, sl])
                        nc.sync.dma_start(out.ap()[dc][:, sl], ob[:])
    nc.compile()
    return nc


# ---------------------------------------------------------------- host side

_NC = None


def _nc_cached():
    global _NC
    if _NC is None:
        _NC = build_nc()
    return _NC


def make_in_maps(query, key_value, Wq, bqv, Wkv, bkvv, Wo):
    _, WMAX, mbias = _plan_cached()
    bf = ml_dtypes.bfloat16
    cstv = np.zeros((1, 512), np.float32)
    cstv[0, 128:256] = 1.0
    mtb_v = np.ascontiguousarray(mbias.astype(ml_dtypes.float8_e4m3))
    idn_v = np.eye(128, dtype=np.float32)
    in_maps = []
    for core in range(8):
        b, h0 = core // 4, 4 * (core % 4)
        cols = slice(h0 * HD, h0 * HD + 256)
        wq_c = (Wq[:, cols] * SCALE).reshape(8, 128, 256).transpose(1, 0, 2)
        wk_c = Wkv[:, h0 * HD:h0 * HD + 256]
        wv_c = Wkv[:, D + h0 * HD:D + h0 * HD + 256]
        wkv_c = np.concatenate([wk_c, wv_c], axis=1)  # [1024, 512]
        wkv_c = wkv_c.reshape(8, 128, 512).transpose(1, 0, 2)
        wo_c = Wo[h0 * HD:h0 * HD + 256, :].reshape(2, 128, 1024).transpose(1, 0, 2)
        bq_c = (bqv[cols] * SCALE).reshape(2, 128).T
        bkv_c = np.concatenate([bkvv[h0 * HD:h0 * HD + 256],
                                bkvv[D + h0 * HD:D + h0 * HD + 256]]).reshape(1, 512)
        in_maps.append({
            "xq": np.ascontiguousarray(query[b].T.astype(np.float32)),
            "xkv": np.ascontiguousarray(key_value[b].T.astype(np.float32)),
            "wq": np.ascontiguousarray(wq_c.astype(np.float32)),
            "wkv": np.ascontiguousarray(wkv_c.astype(np.float32)),
            "wo": np.ascontiguousarray(wo_c.astype(np.float32)),
            "bq": np.ascontiguousarray(bq_c.astype(np.float32)),
            "bkv": bkv_c.astype(np.float32),
            "mtb": mtb_v,
            "cst": cstv,
            "idn": idn_v,
        })
    return in_maps


def assemble(results, bo):
    outs = []
    for b in range(2):
        acc = np.zeros((S, D), np.float64)
        for core in range(b * 4, b * 4 + 4):
            pt = results[core]["out"].reshape(D, S)
            acc += pt.astype(np.float64).T
        outs.append((acc + bo.astype(np.float64)).astype(np.float32))
    return np.stack(outs)


def kernel(query, key_value, Wq, bq, Wkv, bkv, Wo, bo):
    from concourse.bass_utils import run_bass_kernel_spmd
    in_maps = make_in_maps(np.asarray(query, np.float32),
                           np.asarray(key_value, np.float32),
                           np.asarray(Wq, np.float32), np.asarray(bq, np.float32),
                           np.asarray(Wkv, np.float32), np.asarray(bkv, np.float32),
                           np.asarray(Wo, np.float32))
    nc = _nc_cached()
    res = run_bass_kernel_spmd(nc, in_maps, core_ids=list(range(8)), trace=False)
    return assemble(res.results, np.asarray(bo, np.float32))



# revision 9
# speedup vs baseline: 19.8536x; 19.8536x over previous
"""Cantor cross-attention Trainium2 kernel (seq-sharded, bf16, in-kernel KV
all-gather, cached jit executable + device-resident weights).

Sharding: core c = (batch b = c//4, si-quarter q = c%4). Each core computes
all 16 heads' attention for its 512 query rows and emits the final output
slice out[b, 512q:512(q+1), :] directly (no host reduction).

Per-call host->device traffic: query + key_value slices in bf16 (16 MB
total); per-call device->host: output slices in bf16 (8 MB). Weights, the
static Cantor mask table and small constants are device-resident (cached,
re-uploaded only if the weight bytes change). One jit dispatch per call;
output buffers are donated from the previous call.

Dataflow per core (scores kept transposed: [sj partition, si free]):
  xqT/xkvT = PE-transpose of the natural x slices
  qt[g]   = Wq_g^T xqT  (Q^T per 2-head group, scale folded into Wq)
  ktsl[g] = Wk_g^T xkvT (K^T of own sj-slice)  -> all-gather -> kt
  vsl     = xkvT^T Wv   (V natural of own sj-slice) -> all-gather -> vbn|1
  per head h, sj-chunk c: psc = kt_c^T qt (K=64); pb = exp(psc) * mask_c
  psb[65, si] += [V|1]^T pb  (K=128; row 64 = softmax denom)
  oa = psb[0:64] / denom;  out[si, :] = oa^T Wo + bo
"""

import zlib
import numpy as np
import ml_dtypes

import jax
from jax.sharding import Mesh, PartitionSpec, NamedSharding
from jax.experimental.shard_map import shard_map

import concourse.bacc as bacc
import concourse.mybir as mybir
from concourse import tile
import concourse.bass2jax as b2j

F32 = mybir.dt.float32
BF16 = mybir.dt.bfloat16
IDENT = mybir.ActivationFunctionType.Identity
EXP = mybir.ActivationFunctionType.Exp

B, S, D, H, HD = 2, 2048, 1024, 16, 64
SI = 512                # si rows per core
NCH = S // 128          # 16 sj chunks
NG = 8                  # head groups (2 heads of 64 = 128 partitions)
DEPTH, LOCAL_W = 7, 64
SCALE = 1.0 / HD ** 0.5
N_CORES = 8
BF = ml_dtypes.bfloat16


def _cantor_mask():
    idx = np.arange(S)
    d = np.abs(idx[:, None] - idx[None, :])
    x = d.copy()
    ok = np.ones_like(d, dtype=bool)
    for _ in range(DEPTH):
        ok &= (x % 3) != 1
        x //= 3
    ok &= x == 0
    return ok | (d <= LOCAL_W)


# ---------------------------------------------------------------- bass build

def build_nc():
    nc = bacc.Bacc("TRN2", target_bir_lowering=False, debug=False,
                   num_devices=N_CORES)

    xq = nc.dram_tensor("xq", [SI, D], BF16, kind="ExternalInput")
    xkv = nc.dram_tensor("xkv", [SI, D], BF16, kind="ExternalInput")
    wq_d = nc.dram_tensor("wq", [128, 8, D], BF16, kind="ExternalInput")
    wkv_d = nc.dram_tensor("wkv", [128, 8, 2 * D], BF16, kind="ExternalInput")
    wo_d = nc.dram_tensor("wo", [128, 8, D], BF16, kind="ExternalInput")
    bq_d = nc.dram_tensor("bq", [128, 8], F32, kind="ExternalInput")
    bk_d = nc.dram_tensor("bk", [128, 8], F32, kind="ExternalInput")
    bv_d = nc.dram_tensor("bv", [1, D], BF16, kind="ExternalInput")
    bo_d = nc.dram_tensor("bo", [1, D], BF16, kind="ExternalInput")
    mtb_d = nc.dram_tensor("mtb", [128, NCH, SI], BF16, kind="ExternalInput")
    cst_d = nc.dram_tensor("cst", [1, 256], BF16, kind="ExternalInput")
    idn_d = nc.dram_tensor("idn", [128, 128], BF16, kind="ExternalInput")
    dscr = nc.dram_tensor("dscr", [H, SI], F32, kind="Internal")
    out = nc.dram_tensor("out", [SI, D], BF16, kind="ExternalOutput")

    with tile.TileContext(nc) as tc:
        with tc.tile_pool(name="consts", bufs=1) as cp, \
             tc.tile_pool(name="persist", bufs=1) as pp, \
             tc.tile_pool(name="dram", bufs=1, space="DRAM") as dp:
            wq_t = cp.tile([128, 8, D], BF16)
            wo_t = cp.tile([128, 8, D], BF16)
            bq_t = cp.tile([128, 8], F32)
            bk_t = cp.tile([128, 8], F32)
            bv_t = cp.tile([1, D], BF16)
            bo_t = cp.tile([1, D], BF16)
            cst_t = cp.tile([1, 256], BF16)
            idn_t = cp.tile([128, 128], BF16)
            mtb = cp.tile([128, NCH, SI], BF16)
            for dst, src in ((wq_t, wq_d), (wo_t, wo_d), (bq_t, bq_d),
                             (bk_t, bk_d), (bv_t, bv_d), (bo_t, bo_d),
                             (cst_t, cst_d), (idn_t, idn_d), (mtb, mtb_d)):
                nc.sync.dma_start(dst[:], src.ap())
            ones1 = cst_t[0:1, 0:128]   # K=1 lhsT of ones for bias matmuls

            qt = [pp.tile([128, SI], BF16, name=f"qt{g}") for g in range(NG)]
            kt = [pp.tile([128, S], BF16, name=f"kt{g}") for g in range(NG)]
            vbn = [pp.tile([128, H * 65], BF16, name=f"vbn{c}")
                   for c in range(NCH)]
            oa = [pp.tile([128, SI], BF16, name=f"oa{g}") for g in range(NG)]

            ktg_in = dp.tile([8, 128, SI], BF16)       # own K^T slice
            ktg_out = dp.tile([4, 8, 128, SI], BF16)   # gathered K^T
            vg_in = dp.tile([4, 128, D], BF16)         # own V slice (natural)
            vg_out = dp.tile([4, 4, 128, D], BF16)     # gathered V

            # ---- phase A: load + PE-transpose x slices ----
            with tc.tile_pool(name="xpose", bufs=1) as xp, \
                 tc.tile_pool(name="wkvp", bufs=1) as wp:
                wkv_t = wp.tile([128, 8, 2 * D], BF16)
                nc.sync.dma_start(wkv_t[:], wkv_d.ap())
                xqT = [xp.tile([128, SI], BF16, name=f"xqT{dc}")
                       for dc in range(8)]
                xkvT = [xp.tile([128, SI], BF16, name=f"xkvT{dc}")
                        for dc in range(8)]
                with tc.tile_pool(name="xn", bufs=4) as xnp, \
                     tc.tile_pool(name="pt", bufs=4, space="PSUM") as ptp:
                    for src, dstT, nm in ((xq, xqT, "q"), (xkv, xkvT, "kv")):
                        for t in range(4):
                            xn = xnp.tile([128, D], BF16, name=f"xn{nm}{t}",
                                          tag="xn")
                            nc.sync.dma_start(
                                xn[:], src.ap()[t * 128:(t + 1) * 128, :])
                            for dc in range(8):
                                ps = ptp.tile([128, 128], BF16,
                                              name=f"pt{nm}{t}_{dc}", tag="pt")
                                nc.tensor.transpose(
                                    ps[:], xn[:, dc * 128:(dc + 1) * 128],
                                    idn_t[:])
                                dst = dstT[dc][:, t * 128:(t + 1) * 128]
                                if dc % 2 == 0:
                                    nc.vector.tensor_copy(dst, ps[:])
                                else:
                                    nc.scalar.copy(dst, ps[:])

                # ---- phase B: projections of own slices ----
                with tc.tile_pool(name="prj", bufs=3) as prj, \
                     tc.tile_pool(name="pq", bufs=3, space="PSUM") as pqp, \
                     tc.tile_pool(name="pv", bufs=2, space="PSUM") as pvp:
                    for g in range(NG):
                        psq = pqp.tile([128, SI], F32, name=f"psq{g}",
                                       tag="pq")
                        psk = pqp.tile([128, SI], F32, name=f"psk{g}",
                                       tag="pq")
                        for dc in range(8):
                            nc.tensor.matmul(
                                psq[:], wq_t[:, dc, g * 128:(g + 1) * 128],
                                xqT[dc][:], start=(dc == 0), stop=(dc == 7))
                        for dc in range(8):
                            nc.tensor.matmul(
                                psk[:], wkv_t[:, dc, g * 128:(g + 1) * 128],
                                xkvT[dc][:], start=(dc == 0), stop=(dc == 7))
                        nc.scalar.activation(qt[g][:], psq[:], IDENT,
                                             bias=bq_t[:, g:g + 1], scale=1.0)
                        ksl = prj.tile([128, SI], BF16, name=f"ksl{g}",
                                       tag="ksl")
                        nc.scalar.activation(ksl[:], psk[:], IDENT,
                                             bias=bk_t[:, g:g + 1], scale=1.0)
                        nc.sync.dma_start(ktg_in[:][g], ksl[:])
                    for sc in range(4):
                        psv = pvp.tile([128, D], F32, name=f"psv{sc}",
                                       tag="pv")
                        for n in range(2):
                            nsl = slice(n * 512, (n + 1) * 512)
                            for dc in range(8):
                                nc.tensor.matmul(
                                    psv[:, nsl],
                                    xkvT[dc][:, sc * 128:(sc + 1) * 128],
                                    wkv_t[:, dc, D + n * 512:D + (n + 1) * 512],
                                    start=(dc == 0), stop=False)
                            nc.tensor.matmul(psv[:, nsl], ones1,
                                             bv_t[:, nsl],
                                             start=False, stop=True)
                        vsl = prj.tile([128, D], BF16, name=f"vsl{sc}",
                                       tag="vsl")
                        nc.vector.tensor_copy(vsl[:], psv[:])
                        nc.sync.dma_start(vg_in[:][sc], vsl[:])

            # ---- phase C: all-gather K^T and V across the batch group ----
            groups = [[0, 1, 2, 3], [4, 5, 6, 7]]
            nc.gpsimd.collective_compute(
                "AllGather", mybir.AluOpType.bypass, replica_groups=groups,
                ins=[ktg_in.opt()], outs=[ktg_out.opt()])
            nc.gpsimd.collective_compute(
                "AllGather", mybir.AluOpType.bypass, replica_groups=groups,
                ins=[vg_in.opt()], outs=[vg_out.opt()])

            # ---- phase D: assemble kt / vbn from gathered slices ----
            for g in range(NG):
                for j in range(4):
                    nc.sync.dma_start(kt[g][:, j * SI:(j + 1) * SI],
                                      ktg_out[:][j, g])
            for c in range(NCH):
                j, sc = c // 4, c % 4
                nc.sync.dma_start(
                    vbn[c][:].rearrange("p (h e) -> p h e", e=65)[:, :, 0:64],
                    vg_out[:][j, sc].rearrange("p (h e) -> p h e", e=64))
                nc.sync.dma_start(
                    vbn[c][:].rearrange("p (h e) -> p h e", e=65)[:, :, 64:65],
                    cst_d.ap()[0:1, 0:16].to_broadcast((128, 16)))

            # ---- phase E: attention per head ----
            with tc.tile_pool(name="pbp", bufs=6) as pbp, \
                 tc.tile_pool(name="dbp", bufs=1) as dbp, \
                 tc.tile_pool(name="sps", bufs=3, space="PSUM") as sps, \
                 tc.tile_pool(name="bps", bufs=2, space="PSUM") as bps:
                for h in range(H):
                    g, r0 = h // 2, 64 * (h % 2)
                    psb = bps.tile([65, SI], F32, name=f"psb{h}", tag="psb")
                    for c in range(NCH):
                        psc = sps.tile([128, SI], F32, name=f"sc{h}_{c}",
                                       tag="sc")
                        nc.tensor.matmul(
                            psc[:], kt[g][r0:r0 + 64, c * 128:(c + 1) * 128],
                            qt[g][r0:r0 + 64, :], start=True, stop=True)
                        pb = pbp.tile([128, SI], BF16, name=f"pb{h}_{c}",
                                      tag="pb")
                        nc.scalar.activation(pb[:], psc[:], EXP)
                        eng = nc.vector if (h * NCH + c) % 3 != 2 else nc.gpsimd
                        eng.tensor_mul(pb[:], pb[:], mtb[:, c, :])
                        nc.tensor.matmul(psb[:], vbn[c][:, 65 * h:65 * h + 65],
                                         pb[:], start=(c == 0), stop=(c == 15))
                    psb_sb = dbp.tile([65, SI], F32, name=f"pso{h}", tag="pso",
                                      bufs=2)
                    nc.vector.tensor_copy(psb_sb[:], psb[:])
                    nc.sync.dma_start(dscr.ap()[h:h + 1, :], psb_sb[64:65, :])
                    den = dbp.tile([64, SI], F32, name=f"den{h}", tag="den",
                                   bufs=2)
                    nc.sync.dma_start(
                        den[:], dscr.ap()[h:h + 1, :].to_broadcast((64, SI)))
                    nc.vector.reciprocal(den[:], den[:])
                    nc.vector.tensor_mul(oa[g][r0:r0 + 64, :],
                                         psb_sb[0:64, :], den[:])

            # ---- phase F: output projection ----
            with tc.tile_pool(name="osb", bufs=2) as osp, \
                 tc.tile_pool(name="wop", bufs=2, space="PSUM") as wop:
                for t in range(4):
                    pso = wop.tile([128, D], F32, name=f"pso{t}", tag="wo")
                    for n in range(2):
                        nsl = slice(n * 512, (n + 1) * 512)
                        for g in range(NG):
                            nc.tensor.matmul(pso[:, nsl],
                                             oa[g][:, t * 128:(t + 1) * 128],
                                             wo_t[:, g, n * 512:(n + 1) * 512],
                                             start=(g == 0), stop=False)
                        nc.tensor.matmul(pso[:, nsl], ones1, bo_t[:, nsl],
                                         start=False, stop=True)
                    ob = osp.tile([128, D], BF16, name=f"ob{t}", tag="ob")
                    (nc.scalar.copy if t % 2 == 0 else
                     nc.vector.tensor_copy)(ob[:], pso[:])
                    nc.sync.dma_start(out.ap()[t * 128:(t + 1) * 128, :],
                                      ob[:])
    nc.compile()
    return nc


# ---------------------------------------------------------------- runner

class Runner:
    """Cached-jit SPMD runner with device-resident cached inputs."""

    def __init__(self, nc, n_cores=N_CORES):
        b2j.install_neuronx_cc_hook()
        self.nc = nc
        pname = nc.partition_id_tensor.name if nc.partition_id_tensor else None
        in_names, out_names, out_avals = [], [], []
        for alloc in nc.m.functions[0].allocations:
            if not isinstance(alloc, mybir.MemoryLocationSet):
                continue
            name = alloc.memorylocations[0].name
            if alloc.kind == "ExternalInput":
                if name != pname:
                    in_names.append(name)
            elif alloc.kind == "ExternalOutput":
                out_names.append(name)
                out_avals.append(jax.core.ShapedArray(
                    tuple(alloc.tensor_shape), mybir.dt.np(alloc.dtype)))
        self.in_names, self.out_names = in_names, out_names
        n_params, n_outs = len(in_names), len(out_names)
        all_names = in_names + out_names + ([pname] if pname else [])

        def _body(*args):
            operands = list(args)
            if pname is not None:
                operands.append(b2j.partition_id_tensor())
            return tuple(b2j._bass_exec_p.bind(
                *operands, out_avals=tuple(out_avals),
                in_names=tuple(all_names), out_names=tuple(out_names),
                lowering_input_output_aliases=(),
                sim_require_finite=True, sim_require_nnan=True, nc=nc))

        devices = jax.devices()[:n_cores]
        self.mesh = Mesh(np.asarray(devices), ("core",))
        self.sh = NamedSharding(self.mesh, PartitionSpec("core"))
        self.jit = jax.jit(
            shard_map(_body, mesh=self.mesh,
                      in_specs=(PartitionSpec("core"),) * (n_params + n_outs),
                      out_specs=(PartitionSpec("core"),) * n_outs,
                      check_rep=False),
            donate_argnums=tuple(range(n_params, n_params + n_outs)),
            keep_unused=True)
        zshapes = [(n_cores * a.shape[0], *a.shape[1:]) for a in out_avals]
        zdt = [a.dtype for a in out_avals]
        self.make_zeros = jax.jit(
            lambda: tuple(jax.numpy.zeros(s, d) for s, d in zip(zshapes, zdt)),
            out_shardings=tuple(self.sh for _ in zshapes))
        self._donate = None

    def put(self, arr):
        return jax.device_put(arr, self.sh)

    def run(self, named):
        if self._donate is None:
            self._donate = self.make_zeros()
        args = [named[n] for n in self.in_names]
        donate, self._donate = self._donate, None
        outs = self.jit(*args, *donate)
        self._donate = outs
        return dict(zip(self.out_names, outs))


# ---------------------------------------------------------------- host side

_NC = None
_RUNNER = None
_WCACHE = {"key": None, "dev": None}


def _nc_cached():
    global _NC
    if _NC is None:
        _NC = build_nc()
    return _NC


def _runner():
    global _RUNNER
    if _RUNNER is None:
        _RUNNER = Runner(_nc_cached())
    return _RUNNER


def _wkey(ws):
    h = 0
    for w in ws:
        a = np.ascontiguousarray(w)
        h = zlib.crc32(a.view(np.uint8).reshape(-1), h)
    return h


def _prep_static(r, Wq, bq, Wkv, bkv, Wo, bo):
    """Replicated weight/constant arrays -> committed device arrays."""
    wq = np.ascontiguousarray(
        (np.asarray(Wq, np.float32) * SCALE).reshape(8, 128, D)
        .transpose(1, 0, 2)).astype(BF)
    wkv = np.ascontiguousarray(
        np.asarray(Wkv, np.float32).reshape(8, 128, 2 * D)
        .transpose(1, 0, 2)).astype(BF)
    wo = np.ascontiguousarray(
        np.asarray(Wo, np.float32).reshape(8, 128, D)
        .transpose(1, 0, 2)).astype(BF)
    bqv = np.ascontiguousarray(
        (np.asarray(bq, np.float32) * SCALE).reshape(8, 128).T)
    bkvv = np.asarray(bkv, np.float32)
    bk = np.ascontiguousarray(bkvv[:D].reshape(8, 128).T)
    bv = bkvv[D:].reshape(1, D).astype(BF)
    bov = np.asarray(bo, np.float32).reshape(1, D).astype(BF)
    cst = np.zeros((1, 256), BF)
    cst[0, :128] = 1.0
    idn = np.eye(128, dtype=BF)

    mask = _cantor_mask()
    mtb = np.zeros((N_CORES, 128, NCH, SI), BF)
    for core in range(N_CORES):
        q = core % 4
        sub = mask[q * SI:(q + 1) * SI, :]            # [si local, sj global]
        m = sub.T.reshape(NCH, 128, SI)               # [c, p, si]
        mtb[core] = m.transpose(1, 0, 2).astype(BF)

    def rep(a):
        return np.ascontiguousarray(
            np.broadcast_to(a[None], (N_CORES, *a.shape))
            .reshape(N_CORES * a.shape[0], *a.shape[1:]))

    dev = {}
    for name, arr in (("wq", wq), ("wkv", wkv), ("wo", wo), ("bq", bqv),
                      ("bk", bk), ("bv", bv), ("bo", bov), ("cst", cst),
                      ("idn", idn)):
        dev[name] = r.put(rep(arr))
    dev["mtb"] = r.put(mtb.reshape(N_CORES * 128, NCH, SI))
    jax.block_until_ready(list(dev.values()))
    return dev


def kernel(query, key_value, Wq, bq, Wkv, bkv, Wo, bo):
    r = _runner()
    key = _wkey([Wq, bq, Wkv, bkv, Wo, bo])
    if _WCACHE["key"] != key:
        _WCACHE["dev"] = _prep_static(r, Wq, bq, Wkv, bkv, Wo, bo)
        _WCACHE["key"] = key

    xq = np.asarray(query, np.float32).astype(BF).reshape(N_CORES * SI, D)
    xkv = np.asarray(key_value, np.float32).astype(BF).reshape(
        N_CORES * SI, D)
    named = dict(_WCACHE["dev"])
    named["xq"] = r.put(xq)
    named["xkv"] = r.put(xkv)
    outs = r.run(named)
    res = np.asarray(outs["out"])
    return res.astype(np.float32).reshape(B, S, D)
